# revision 16
# baseline (speedup 1.0000x reference)
"""Trainium2 Bass kernel for nn_BQuantConv1d.

Math (reference):
    sign[k,f,8g+j] = 2*bit_{7-j}(binary[k,f,g]) - 1
    W[f,n]  = sum_k scale[k,f] * sign[k,f,n]          (NF=4096, NX=1024)
    out     = x @ W.T + bias                          (x: (2,2048,1024))

Sharding: NF split across 8 cores (512 features each); x replicated.

Per-core plan (all-bf16 datapath, fp32 PSUM accumulation):
  Decode (PE): for each bit position j (shift s=7-j), extract bits of all
    4 f-tiles with one wide DVE tensor_scalar (b>>s)&1 -> int32, cast to
    bf16 on ScalarE, then matmul bits.T @ diag(2*scale_k) accumulating into
    PSUM. This transposes f->partition, applies the per-(k,f) scale and sums
    the 8 bit-planes in one PSUM group. ones @ diag(-C) (C = sum_k scale,
    bf16 hi+lo split) initializes the group, yielding W.T[8g+j, f].
  W.T rows are scattered (partition-stride-8 SBUF->SBUF DMA) into
    n-contiguous chunks BT[128c+p, f] (bf16).
  GEMM: x is DMA-loaded with an inline fp32->bf16 cast (SWDGE). Per
    128-token tile, PE-transpose the 8 n-chunks (bf16, 1 cyc/row), then
    out_psum = ones.T@bias + sum_c xT_c.T @ BT_c   (bf16 MMs, fp32 PSUM).
"""

import sys

sys.path.insert(0, "/opt/trn_rl_repo")

import numpy as np
import concourse.bass as bass
import concourse.mybir as mybir
import concourse.tile as tile
from concourse import bacc
from concourse.bass_utils import run_bass_kernel_spmd
from concourse.masks import make_identity

F32 = mybir.dt.float32
F32R = mybir.dt.float32r
BF16 = mybir.dt.bfloat16
I32 = mybir.dt.int32
U8 = mybir.dt.uint8
Alu = mybir.AluOpType
Ax = mybir.AxisListType

# Walrus rejects bass-generated explicit InstLdweights when its LDW
# optimization pass is on ("not compatible with LDW optimization"), so this
# must stay False.
_LDW_OPT = False


def _patch_ldw_opt():
    from concourse import bass_utils as bu

    if getattr(bu, "_ldw_patched", False):
        return
    orig = bu.run_command

    def patched(cmd, **kw):
        cmd = [
            "--enable-ldw-opt=true" if c == "--enable-ldw-opt=false" else c
            for c in cmd
        ]
        return orig(cmd, **kw)

    bu.run_command = patched
    bu._ldw_patched = True

NCORES = 8
T = 4096  # tokens (2*2048)
NX = 1024
KB = 8  # bit planes
G = 128  # packed groups per row (NX/8)
NFL = 512  # features per core (4096/8)
NFT = NFL // 128  # f-tiles per core = 4
TT = T // 128  # token tiles = 32
NC = NX // 128  # contraction chunks = 8

_CACHED = {}


def _build_nc():
    if _LDW_OPT:
        _patch_ldw_opt()
    nc = bacc.Bacc(None, target_bir_lowering=False, debug=False)

    x_d = nc.dram_tensor("x", [T, NX], F32, kind="ExternalInput")
    bin_d = nc.dram_tensor("binary", [KB, NFL, G], I32, kind="ExternalInput")
    scale_d = nc.dram_tensor("scale", [KB, NFL], F32, kind="ExternalInput")
    bias_d = nc.dram_tensor("bias", [1, NFL], F32, kind="ExternalInput")
    out_d = nc.dram_tensor("out", [T, NFL], F32, kind="ExternalOutput")

    NDC = NFT * KB + 2 * NFT  # 40 diag blocks

    with tile.TileContext(nc) as tc:
        with (
            tc.tile_pool(name="const", bufs=1) as cpool,
            tc.tile_pool(name="x_sb", bufs=6) as xpool,
            tc.tile_pool(name="xt_sb", bufs=11) as xtpool,
            tc.tile_pool(name="out_sb", bufs=3) as opool,
            tc.tile_pool(name="bits", bufs=2) as bpool,
            tc.tile_pool(name="bt_sb", bufs=2) as btpool,
            tc.tile_pool(name="dec_ps", bufs=2, space="PSUM") as dps,
            tc.tile_pool(name="xt_ps", bufs=2, space="PSUM") as xtps,
            tc.tile_pool(name="out_ps", bufs=2, space="PSUM") as ops,
            tc.tile_pool(name="ps_setup", bufs=1, space="PSUM") as pss,
        ):
            # small setup loads first on the HWDGE queue (they gate the
            # D-matrix build chain), then binary (gates bit extraction).
            scale_sb = cpool.tile([KB, NFL], F32)
            nc.sync.dma_start(scale_sb, scale_d[:, :])
            bias_f = cpool.tile([1, NFL], F32)
            nc.sync.dma_start(bias_f, bias_d[:, :])
            byts_i = cpool.tile([128, NFT * KB * G], I32)
            byts_iv = byts_i.rearrange("f (t k g) -> f t k g", t=NFT, k=KB)
            for ft in range(NFT):
                src = bin_d[:, ft * 128 : (ft + 1) * 128, :].rearrange(
                    "k f g -> f k g"
                )
                nc.sync.dma_start(byts_iv[:, ft], src)
            byts = cpool.tile([128, NFT * KB * G], U8)
            nc.vector.tensor_copy(byts, byts_i)

            # x prefetch: SWDGE DMA with inline fp32->bf16 cast, 4 t-tiles
            # per transfer to amortize descriptor generation.
            XB = 4
            x_tiles = []
            for tb in range(TT // XB):
                x_bf = xpool.tile([128, XB, NX], BF16, name=f"x_bf{tb}", tag="x_bf")
                src = x_d[tb * XB * 128 : (tb + 1) * XB * 128, :].rearrange(
                    "(a p) n -> p a n", a=XB
                )
                nc.gpsimd.dma_start(x_bf, src)
                x_tiles.append(x_bf)

            ident = cpool.tile([128, 128], F32)
            make_identity(nc, ident)
            ident_bf = cpool.tile([128, 128], BF16)
            nc.vector.tensor_copy(ident_bf, ident)
            ones_bf = cpool.tile([128, 128], BF16)
            nc.vector.memset(ones_bf, 1.0)
            ones_row = cpool.tile([1, 128], BF16)
            nc.vector.memset(ones_row, 1.0)
            bias_bf = cpool.tile([1, NFL], BF16)
            nc.vector.tensor_copy(bias_bf, bias_f)

            # ---- scale prep: scaleT[f_part, (ft,k)], C columns, D matrices
            scaleT = cpool.tile([128, NFT * KB], F32)
            for ft in range(NFT):
                ps_t = pss.tile([128, KB], F32, tag="ps_t")
                nc.tensor.transpose(
                    ps_t, scale_sb[:, ft * 128 : (ft + 1) * 128], ident[0:KB, 0:KB]
                )
                nc.vector.tensor_copy(scaleT[:, ft * KB : (ft + 1) * KB], ps_t)

            # negC = -sum_k scale (per f), bf16 hi + lo split
            negC = cpool.tile([128, NFT], F32)
            for ft in range(NFT):
                nc.vector.tensor_reduce(
                    negC[:, ft : ft + 1],
                    scaleT[:, ft * KB : (ft + 1) * KB],
                    axis=Ax.X,
                    op=Alu.add,
                    negate=True,
                )
            negC_hi_bf = cpool.tile([128, NFT], BF16)
            nc.vector.tensor_copy(negC_hi_bf, negC)
            negC_hi_f = cpool.tile([128, NFT], F32)
            nc.vector.tensor_copy(negC_hi_f, negC_hi_bf)
            negC_lo = cpool.tile([128, NFT], F32)
            nc.vector.tensor_sub(negC_lo, negC, negC_hi_f)

            # D[(ft,k)] = diag(2*scale) in f32 -> bulk bf16 cast per half
            D_f = cpool.tile([128, NDC * 128], F32)
            D = cpool.tile([128, NDC * 128], BF16)
            for ft in range(NFT):
                for k in range(KB):
                    nc.vector.tensor_scalar(
                        D_f[:, (ft * KB + k) * 128 : (ft * KB + k + 1) * 128],
                        ident,
                        scaleT[:, ft * KB + k : ft * KB + k + 1],
                        2.0,
                        op0=Alu.mult,
                        op1=Alu.mult,
                    )
            for ft in range(NFT):
                o = (NFT * KB + ft) * 128
                nc.vector.tensor_scalar(
                    D_f[:, o : o + 128],
                    ident,
                    negC_hi_f[:, ft : ft + 1],
                    None,
                    op0=Alu.mult,
                )
                o = (NFT * KB + NFT + ft) * 128
                nc.vector.tensor_scalar(
                    D_f[:, o : o + 128],
                    ident,
                    negC_lo[:, ft : ft + 1],
                    None,
                    op0=Alu.mult,
                )
            half = NDC * 128 // 2
            nc.scalar.copy(D[:, :half], D_f[:, :half])
            nc.scalar.copy(D[:, half:], D_f[:, half:])

            def D_blk(ft, k):
                return D[:, (ft * KB + k) * 128 : (ft * KB + k + 1) * 128]

            def Dc_hi(ft):
                o = (NFT * KB + ft) * 128
                return D[:, o : o + 128]

            def Dc_lo(ft):
                o = (NFT * KB + NFT + ft) * 128
                return D[:, o : o + 128]

            # bias broadcast tile [128, NFL] via rank-1 ones matmul
            bias_bc = cpool.tile([128, NFL], F32)
            ps_b = pss.tile([128, NFL], F32, tag="ps_b")
            nc.tensor.matmul(ps_b, ones_row, bias_bf, start=True, stop=True)
            nc.vector.tensor_copy(bias_bc, ps_b)

            # ---- full W.T in n-contiguous chunk layout: BT[p, c, f] (bf16)
            BT = cpool.tile([128, NC, NFL], BF16)
            BT_j = BT.rearrange("(gl j) c f -> j gl c f", j=8)

            # ================= decode + transposes interleaved =========
            # PE engine queue is FIFO: interleave independent transpose
            # work between decode blocks so bit-extract/cast latency never
            # leaves the PE idle.
            xt_tiles = {}

            def transpose_block(tt):
                x_bf = x_tiles[tt // XB][:, tt % XB, :]
                xt_ps = xtps.tile([128, NC * 128], BF16, name=f"xtp{tt}", tag="xt_ps")
                for c in range(NC):
                    nc.tensor.transpose(
                        xt_ps[:, c * 128 : (c + 1) * 128],
                        x_bf[:, c * 128 : (c + 1) * 128],
                        ident_bf,
                    )
                xt_sb = xtpool.tile(
                    [128, NC, 128], BF16, name=f"xt{tt}", tag="xt_sb"
                )
                if tt % 2 == 0:
                    nc.vector.tensor_copy(xt_sb, xt_ps)
                else:
                    nc.scalar.copy(xt_sb, xt_ps)
                xt_tiles[tt] = xt_sb

            def decode_block(j):
                s = 7 - j
                psum_j = dps.tile([128, NFL], F32, name=f"psj{j}", tag="psum_j")
                bits_bf = []
                for h in range(2):  # halves: ft {0,1} and {2,3}
                    hs = slice(h * 2 * KB * G, (h + 1) * 2 * KB * G)
                    bu = bpool.tile(
                        [128, 2 * KB * G], U8, name=f"bu{j}_{h}", tag=f"bits_u{h}"
                    )
                    nc.vector.tensor_scalar(
                        bu,
                        byts[:, hs],
                        s,
                        1,
                        op0=Alu.logical_shift_right,
                        op1=Alu.bitwise_and,
                    )
                    bb = bpool.tile(
                        [128, 2 * KB * G], BF16, name=f"bb{j}_{h}", tag=f"bits_bf{h}"
                    )
                    nc.scalar.copy(bb, bu)
                    bits_bf.append(bb)
                for ft in range(NFT):
                    blk = slice(ft * 128, (ft + 1) * 128)
                    bb = bits_bf[ft // 2]
                    off = (ft % 2) * KB * G
                    nc.tensor.matmul(
                        psum_j[:, blk], ones_bf, Dc_hi(ft), start=True, stop=False
                    )
                    nc.tensor.matmul(
                        psum_j[:, blk], ones_bf, Dc_lo(ft), start=False, stop=False
                    )
                    for k in range(KB):
                        nc.tensor.matmul(
                            psum_j[:, blk],
                            bb[:, off + k * G : off + (k + 1) * G],
                            D_blk(ft, k),
                            start=False,
                            stop=(k == KB - 1),
                        )
                btj = btpool.tile([128, NFL], BF16, name=f"btj{j}", tag="btj")
                if j % 2 == 0:
                    nc.vector.tensor_copy(btj, psum_j)
                else:
                    nc.scalar.copy(btj, psum_j)
                # scatter rows g -> partitions 8*(g%16)+j, chunk g//16
                for c in range(NC):
                    nc.sync.dma_start(BT_j[j][:, c, :], btj[c * 16 : (c + 1) * 16, :])

            def gemm_block(tt):
                xt_sb = xt_tiles.pop(tt)
                out_ps = ops.tile([128, NFL], F32, name=f"op{tt}", tag="out_ps")
                for c in range(NC):
                    nc.tensor.matmul(
                        out_ps,
                        xt_sb[:, c, :],
                        BT[:, c, :],
                        start=(c == 0),
                        stop=(c == NC - 1),
                    )
                out_sb = opool.tile([128, NFL], F32, name=f"os{tt}", tag="out_sb")
                nc.vector.tensor_add(out_sb, out_ps, bias_bc)
                nc.sync.dma_start(out_d[tt * 128 : (tt + 1) * 128, :], out_sb)

            PRE = 10  # transposes interleaved into the decode phase
            for j in range(8):
                decode_block(j)
                transpose_block(j)
            transpose_block(8)
            transpose_block(9)
            for tt in range(TT):
                gemm_block(tt)
                if tt + PRE < TT:
                    transpose_block(tt + PRE)

    nc.finalize()
    return nc


def _install_ntff_hook():
    """The agent image's antenv lacks axon_hooks; synthesize it so
    run_bass_kernel_spmd(trace=True) can capture NTFF profiles."""
    import types

    if "antenv.axon_hooks" in sys.modules:
        return
    import antenv
    from trn_agent_boot.trn_boot import _ntff_profile_via_ctypes

    mod = types.ModuleType("antenv.axon_hooks")
    state = {"hook": _ntff_profile_via_ctypes("/opt/axon/libaxon_pjrt.so")}
    mod.set_axon_ntff_profile_hook = lambda h: state.__setitem__("hook", h)
    mod.get_axon_ntff_profile_hook = lambda: state["hook"]
    sys.modules["antenv.axon_hooks"] = mod
    antenv.axon_hooks = mod


def kernel(x, binary, scale, bias, _trace=False):
    x = np.ascontiguousarray(np.asarray(x), dtype=np.float32)
    binary = np.ascontiguousarray(np.asarray(binary), dtype=np.int32)
    scale = np.ascontiguousarray(np.asarray(scale), dtype=np.float32)
    bias = np.ascontiguousarray(np.asarray(bias), dtype=np.float32)

    orig_shape = x.shape[:-1] + (binary.shape[1],)
    xf = x.reshape(-1, x.shape[-1])

    if "nc" not in _CACHED:
        _CACHED["nc"] = _build_nc()
    nc = _CACHED["nc"]

    in_maps = []
    for i in range(NCORES):
        fsl = slice(i * NFL, (i + 1) * NFL)
        in_maps.append(
            {
                "x": xf,
                "binary": binary[:, fsl, :],
                "scale": scale[:, fsl, 0] if scale.ndim == 3 else scale[:, fsl],
                "bias": bias[fsl].reshape(1, NFL),
            }
        )

    kw = {}
    if _trace:
        _install_ntff_hook()
        kw = dict(trace=True, trace_cores=[0])
    res = run_bass_kernel_spmd(nc, in_maps, core_ids=list(range(NCORES)), **kw)
    out = np.concatenate([res.results[i]["out"] for i in range(NCORES)], axis=1)
    if _trace:
        return out.reshape(orig_shape), res
    return out.reshape(orig_shape)


# revision 17
# speedup vs baseline: 1.1909x; 1.1909x over previous
"""Trainium2 Bass kernel for nn_BQuantConv1d.

Math (reference):
    sign[k,f,8g+j] = 2*bit_{7-j}(binary[k,f,g]) - 1
    W[f,n]  = sum_k scale[k,f] * sign[k,f,n]          (NF=4096, NX=1024)
    out     = x @ W.T + bias                          (x: (2,2048,1024))

Sharding: NF split across 8 cores (512 features each); x replicated.

Per-core plan (all-bf16 datapath, fp32 PSUM accumulation):
  Decode (PE): for each bit position j (shift s=7-j), extract bits of all
    4 f-tiles with one wide DVE tensor_scalar (b>>s)&1 -> int32, cast to
    bf16 on ScalarE, then matmul bits.T @ diag(2*scale_k) accumulating into
    PSUM. This transposes f->partition, applies the per-(k,f) scale and sums
    the 8 bit-planes in one PSUM group. ones @ diag(-C) (C = sum_k scale,
    bf16 hi+lo split) initializes the group, yielding W.T[8g+j, f].
  W.T rows are scattered (partition-stride-8 SBUF->SBUF DMA) into
    n-contiguous chunks BT[128c+p, f] (bf16).
  GEMM: x is DMA-loaded with an inline fp32->bf16 cast (SWDGE). Per
    128-token tile, PE-transpose the 8 n-chunks (bf16, 1 cyc/row), then
    out_psum = ones.T@bias + sum_c xT_c.T @ BT_c   (bf16 MMs, fp32 PSUM).
"""

import sys

sys.path.insert(0, "/opt/trn_rl_repo")

import numpy as np
import concourse.bass as bass
import concourse.mybir as mybir
import concourse.tile as tile
from concourse import bacc
from concourse.bass_utils import run_bass_kernel_spmd
from concourse.masks import make_identity

F32 = mybir.dt.float32
F32R = mybir.dt.float32r
BF16 = mybir.dt.bfloat16
I32 = mybir.dt.int32
U8 = mybir.dt.uint8
Alu = mybir.AluOpType
Ax = mybir.AxisListType

# Walrus rejects bass-generated explicit InstLdweights when its LDW
# optimization pass is on ("not compatible with LDW optimization"), so this
# must stay False.
_LDW_OPT = False


def _patch_ldw_opt():
    from concourse import bass_utils as bu

    if getattr(bu, "_ldw_patched", False):
        return
    orig = bu.run_command

    def patched(cmd, **kw):
        cmd = [
            "--enable-ldw-opt=true" if c == "--enable-ldw-opt=false" else c
            for c in cmd
        ]
        return orig(cmd, **kw)

    bu.run_command = patched
    bu._ldw_patched = True

NCORES = 8
T = 4096  # tokens (2*2048)
NX = 1024
KB = 8  # bit planes
G = 128  # packed groups per row (NX/8)
NFL = 512  # features per core (4096/8)
NFT = NFL // 128  # f-tiles per core = 4
TT = T // 128  # token tiles = 32
NC = NX // 128  # contraction chunks = 8

_CACHED = {}


def _build_nc():
    if _LDW_OPT:
        _patch_ldw_opt()
    nc = bacc.Bacc(None, target_bir_lowering=False, debug=False)

    x_d = nc.dram_tensor("x", [T, NX], F32, kind="ExternalInput")
    bin_d = nc.dram_tensor("binary", [KB, NFL, G], I32, kind="ExternalInput")
    scale_d = nc.dram_tensor("scale", [KB, NFL], F32, kind="ExternalInput")
    bias_d = nc.dram_tensor("bias", [1, NFL], F32, kind="ExternalInput")
    out_d = nc.dram_tensor("out", [T, NFL], F32, kind="ExternalOutput")

    NDC = NFT * KB + 2 * NFT  # 40 diag blocks

    with tile.TileContext(nc) as tc:
        with (
            tc.tile_pool(name="const", bufs=1) as cpool,
            tc.tile_pool(name="x_sb", bufs=6) as xpool,
            tc.tile_pool(name="xt_sb", bufs=11) as xtpool,
            tc.tile_pool(name="out_sb", bufs=3) as opool,
            tc.tile_pool(name="bits", bufs=2) as bpool,
            tc.tile_pool(name="bt_sb", bufs=2) as btpool,
            tc.tile_pool(name="dec_ps", bufs=2, space="PSUM") as dps,
            tc.tile_pool(name="xt_ps", bufs=2, space="PSUM") as xtps,
            tc.tile_pool(name="out_ps", bufs=2, space="PSUM") as ops,
            tc.tile_pool(name="ps_setup", bufs=1, space="PSUM") as pss,
        ):
            # packed sign bytes: SWDGE cast-DMA int32->uint8 (values < 256)
            byts = cpool.tile([128, NFT * KB * G], U8)
            byts_v = byts.rearrange("f (t k g) -> f t k g", t=NFT, k=KB)
            for ft in range(NFT):
                src = bin_d[:, ft * 128 : (ft + 1) * 128, :].rearrange(
                    "k f g -> f k g"
                )
                nc.gpsimd.dma_start(byts_v[:, ft], src)

            # x prefetch: SWDGE DMA with inline fp32->bf16 cast, 4 t-tiles
            # per transfer to amortize descriptor generation.
            XB = 4
            x_tiles = []
            for tb in range(TT // XB):
                x_bf = xpool.tile([128, XB, NX], BF16, name=f"x_bf{tb}", tag="x_bf")
                src = x_d[tb * XB * 128 : (tb + 1) * XB * 128, :].rearrange(
                    "(a p) n -> p a n", a=XB
                )
                nc.gpsimd.dma_start(x_bf, src)
                x_tiles.append(x_bf)

            ident = cpool.tile([128, 128], F32)
            make_identity(nc, ident)
            ident_bf = cpool.tile([128, 128], BF16)
            nc.vector.tensor_copy(ident_bf, ident)
            ones_bf = cpool.tile([128, 128], BF16)
            nc.vector.memset(ones_bf, 1.0)
            ones_row = cpool.tile([1, 128], BF16)
            nc.vector.memset(ones_row, 1.0)
            bias_f = cpool.tile([1, NFL], F32)
            nc.sync.dma_start(bias_f, bias_d[:, :])
            bias_bf = cpool.tile([1, NFL], BF16)
            nc.vector.tensor_copy(bias_bf, bias_f)

            # ---- scale prep: scaleT[f_part, (ft,k)], C columns, D matrices
            scale_sb = cpool.tile([KB, NFL], F32)
            nc.sync.dma_start(scale_sb, scale_d[:, :])
            scaleT = cpool.tile([128, NFT * KB], F32)
            for ft in range(NFT):
                ps_t = pss.tile([128, KB], F32, tag="ps_t")
                nc.tensor.transpose(
                    ps_t, scale_sb[:, ft * 128 : (ft + 1) * 128], ident[0:KB, 0:KB]
                )
                nc.vector.tensor_copy(scaleT[:, ft * KB : (ft + 1) * KB], ps_t)

            # negC = -sum_k scale (per f), bf16 hi + lo split
            negC = cpool.tile([128, NFT], F32)
            for ft in range(NFT):
                nc.vector.tensor_reduce(
                    negC[:, ft : ft + 1],
                    scaleT[:, ft * KB : (ft + 1) * KB],
                    axis=Ax.X,
                    op=Alu.add,
                    negate=True,
                )
            negC_hi_bf = cpool.tile([128, NFT], BF16)
            nc.vector.tensor_copy(negC_hi_bf, negC)
            negC_hi_f = cpool.tile([128, NFT], F32)
            nc.vector.tensor_copy(negC_hi_f, negC_hi_bf)
            negC_lo = cpool.tile([128, NFT], F32)
            nc.vector.tensor_sub(negC_lo, negC, negC_hi_f)

            # D[(ft,k)] = diag(2*scale) in f32 -> bulk bf16 cast per half
            D_f = cpool.tile([128, NDC * 128], F32)
            D = cpool.tile([128, NDC * 128], BF16)
            for ft in range(NFT):
                for k in range(KB):
                    nc.vector.tensor_scalar(
                        D_f[:, (ft * KB + k) * 128 : (ft * KB + k + 1) * 128],
                        ident,
                        scaleT[:, ft * KB + k : ft * KB + k + 1],
                        2.0,
                        op0=Alu.mult,
                        op1=Alu.mult,
                    )
            for ft in range(NFT):
                o = (NFT * KB + ft) * 128
                nc.vector.tensor_scalar(
                    D_f[:, o : o + 128],
                    ident,
                    negC_hi_f[:, ft : ft + 1],
                    None,
                    op0=Alu.mult,
                )
                o = (NFT * KB + NFT + ft) * 128
                nc.vector.tensor_scalar(
                    D_f[:, o : o + 128],
                    ident,
                    negC_lo[:, ft : ft + 1],
                    None,
                    op0=Alu.mult,
                )
            half = NDC * 128 // 2
            nc.scalar.copy(D[:, :half], D_f[:, :half])
            nc.scalar.copy(D[:, half:], D_f[:, half:])

            def D_blk(ft, k):
                return D[:, (ft * KB + k) * 128 : (ft * KB + k + 1) * 128]

            def Dc_hi(ft):
                o = (NFT * KB + ft) * 128
                return D[:, o : o + 128]

            def Dc_lo(ft):
                o = (NFT * KB + NFT + ft) * 128
                return D[:, o : o + 128]

            # bias broadcast tile [128, NFL] via rank-1 ones matmul
            bias_bc = cpool.tile([128, NFL], F32)
            ps_b = pss.tile([128, NFL], F32, tag="ps_b")
            nc.tensor.matmul(ps_b, ones_row, bias_bf, start=True, stop=True)
            nc.vector.tensor_copy(bias_bc, ps_b)

            # ---- full W.T in n-contiguous chunk layout: BT[p, c, f] (bf16)
            BT = cpool.tile([128, NC, NFL], BF16)
            BT_j = BT.rearrange("(gl j) c f -> j gl c f", j=8)

            # ================= decode + transposes interleaved =========
            # PE engine queue is FIFO: interleave independent transpose
            # work between decode blocks so bit-extract/cast latency never
            # leaves the PE idle.
            xt_tiles = {}

            def transpose_block(tt):
                x_bf = x_tiles[tt // XB][:, tt % XB, :]
                xt_ps = xtps.tile([128, NC * 128], BF16, name=f"xtp{tt}", tag="xt_ps")
                for c in range(NC):
                    nc.tensor.transpose(
                        xt_ps[:, c * 128 : (c + 1) * 128],
                        x_bf[:, c * 128 : (c + 1) * 128],
                        ident_bf,
                    )
                xt_sb = xtpool.tile(
                    [128, NC, 128], BF16, name=f"xt{tt}", tag="xt_sb"
                )
                if tt % 2 == 0:
                    nc.vector.tensor_copy(xt_sb, xt_ps)
                else:
                    nc.scalar.copy(xt_sb, xt_ps)
                xt_tiles[tt] = xt_sb

            def decode_block(j):
                s = 7 - j
                psum_j = dps.tile([128, NFL], F32, name=f"psj{j}", tag="psum_j")
                bits_bf = []
                for h in range(2):  # halves: ft {0,1} and {2,3}
                    hs = slice(h * 2 * KB * G, (h + 1) * 2 * KB * G)
                    bu = bpool.tile(
                        [128, 2 * KB * G], U8, name=f"bu{j}_{h}", tag=f"bits_u{h}"
                    )
                    nc.vector.tensor_scalar(
                        bu,
                        byts[:, hs],
                        s,
                        1,
                        op0=Alu.logical_shift_right,
                        op1=Alu.bitwise_and,
                    )
                    bb = bpool.tile(
                        [128, 2 * KB * G], BF16, name=f"bb{j}_{h}", tag=f"bits_bf{h}"
                    )
                    nc.scalar.copy(bb, bu)
                    bits_bf.append(bb)
                for ft in range(NFT):
                    blk = slice(ft * 128, (ft + 1) * 128)
                    bb = bits_bf[ft // 2]
                    off = (ft % 2) * KB * G
                    nc.tensor.matmul(
                        psum_j[:, blk], ones_bf, Dc_hi(ft), start=True, stop=False
                    )
                    nc.tensor.matmul(
                        psum_j[:, blk], ones_bf, Dc_lo(ft), start=False, stop=False
                    )
                    for k in range(KB):
                        nc.tensor.matmul(
                            psum_j[:, blk],
                            bb[:, off + k * G : off + (k + 1) * G],
                            D_blk(ft, k),
                            start=False,
                            stop=(k == KB - 1),
                        )
                btj = btpool.tile([128, NFL], BF16, name=f"btj{j}", tag="btj")
                if j % 2 == 0:
                    nc.vector.tensor_copy(btj, psum_j)
                else:
                    nc.scalar.copy(btj, psum_j)
                # scatter rows g -> partitions 8*(g%16)+j, chunk g//16
                for c in range(NC):
                    nc.sync.dma_start(BT_j[j][:, c, :], btj[c * 16 : (c + 1) * 16, :])

            def gemm_block(tt):
                xt_sb = xt_tiles.pop(tt)
                out_ps = ops.tile([128, NFL], F32, name=f"op{tt}", tag="out_ps")
                for c in range(NC):
                    nc.tensor.matmul(
                        out_ps,
                        xt_sb[:, c, :],
                        BT[:, c, :],
                        start=(c == 0),
                        stop=(c == NC - 1),
                    )
                out_sb = opool.tile([128, NFL], F32, name=f"os{tt}", tag="out_sb")
                nc.vector.tensor_add(out_sb, out_ps, bias_bc)
                nc.sync.dma_start(out_d[tt * 128 : (tt + 1) * 128, :], out_sb)

            PRE = 10  # transposes interleaved into the decode phase
            for j in range(8):
                decode_block(j)
                transpose_block(j)
            transpose_block(8)
            transpose_block(9)
            for tt in range(TT):
                gemm_block(tt)
                if tt + PRE < TT:
                    transpose_block(tt + PRE)

    nc.finalize()
    return nc


def _install_ntff_hook():
    """The agent image's antenv lacks axon_hooks; synthesize it so
    run_bass_kernel_spmd(trace=True) can capture NTFF profiles."""
    import types

    if "antenv.axon_hooks" in sys.modules:
        return
    import antenv
    from trn_agent_boot.trn_boot import _ntff_profile_via_ctypes

    mod = types.ModuleType("antenv.axon_hooks")
    state = {"hook": _ntff_profile_via_ctypes("/opt/axon/libaxon_pjrt.so")}
    mod.set_axon_ntff_profile_hook = lambda h: state.__setitem__("hook", h)
    mod.get_axon_ntff_profile_hook = lambda: state["hook"]
    sys.modules["antenv.axon_hooks"] = mod
    antenv.axon_hooks = mod


def kernel(x, binary, scale, bias, _trace=False):
    x = np.ascontiguousarray(np.asarray(x), dtype=np.float32)
    binary = np.ascontiguousarray(np.asarray(binary), dtype=np.int32)
    scale = np.ascontiguousarray(np.asarray(scale), dtype=np.float32)
    bias = np.ascontiguousarray(np.asarray(bias), dtype=np.float32)

    orig_shape = x.shape[:-1] + (binary.shape[1],)
    xf = x.reshape(-1, x.shape[-1])

    if "nc" not in _CACHED:
        _CACHED["nc"] = _build_nc()
    nc = _CACHED["nc"]

    in_maps = []
    for i in range(NCORES):
        fsl = slice(i * NFL, (i + 1) * NFL)
        in_maps.append(
            {
                "x": xf,
                "binary": binary[:, fsl, :],
                "scale": scale[:, fsl, 0] if scale.ndim == 3 else scale[:, fsl],
                "bias": bias[fsl].reshape(1, NFL),
            }
        )

    kw = {}
    if _trace:
        _install_ntff_hook()
        kw = dict(trace=True, trace_cores=[0])
    res = run_bass_kernel_spmd(nc, in_maps, core_ids=list(range(NCORES)), **kw)
    out = np.concatenate([res.results[i]["out"] for i in range(NCORES)], axis=1)
    if _trace:
        return out.reshape(orig_shape), res
    return out.reshape(orig_shape)
